# revision 21
# baseline (speedup 1.0000x reference)
"""Trainium2 Bass kernel for AdaptiveRankFusionLayer (CP low-rank fusion).

    out = ((x1 @ f1) * (x2 @ f2) * (x3 @ f3)) @ f_out.T

Data-parallel batch sharding across 8 NeuronCores (65536 -> 8192
rows/core), no collectives.

v2 design (vs the all-DMA-bound v1 at ~290-313 us):
  * fp16 I/O in DRAM: the host casts x1/x2/x3 (and factors) f32->fp16
    before upload and upcasts the fp16 output after readback. This
    halves HBM traffic per core from ~92 MB to ~46 MB. v1's trace
    showed DMA 90.9% active (input SWDGE 257 us of 287 us wall), so
    traffic is the binding constraint. fp16 (not bf16) for the extra
    mantissa headroom; rel err ~2e-3, gate is 2e-2.
  * Reoriented rank matmuls: after the PE transpose of each 128x128 x
    block, the transposed tile is used as the STATIONARY operand and
    the tiny factor f_i [128,10] streams as the moving operand (10
    columns instead of 512). y lands in natural [batch, rank] layout
    in PSUM; the Hadamard product happens there; only the tiny
    [128,10] result is transposed back for the final matmul.
    PE time drops ~14.9k cycles/supertile (~99 us/core) vs ~20.9k.
  * PSUM->SBUF copies rotate across DVE / Act / Pool so no single
    drain engine is critical.
  * Software-pipelined emission: k-matmuls lag their transpose group
    by 2 so the PE never waits on a copy round-trip; each supertile's
    tail (Hadamard transpose + final matmuls + store) is emitted
    inside the next supertile's transpose stream.

Layout: batch rows -> (s p blk) so each partition's DMA segment is 4
consecutive DRAM rows (fully contiguous, 2-8 KB per partition per
supertile); the same permutation is applied on the store side so the
external layout is exact.
"""

import itertools
import sys
import types

import numpy as np

import concourse.bass as bass
import concourse.mybir as mybir
import concourse.tile as tile
from concourse import bacc
from concourse.bass_utils import run_bass_kernel_spmd
from concourse.masks import make_identity


def _install_profile_shim():
    """Make trace=True / BASS_TRACE=1 work in this container: provide the
    antenv.axon_hooks module the axon NTFF-profile path imports, and make
    artifact upload a no-op (no object store here). Safe no-op if the real
    module exists."""
    try:
        if "antenv.axon_hooks" not in sys.modules:
            try:
                import antenv.axon_hooks  # noqa: F401
            except ImportError:
                mod = types.ModuleType("antenv.axon_hooks")
                mod._hook = None
                mod.set_axon_ntff_profile_hook = (
                    lambda h: setattr(mod, "_hook", h))
                mod.get_axon_ntff_profile_hook = lambda: mod._hook
                sys.modules["antenv.axon_hooks"] = mod
                import antenv
                antenv.axon_hooks = mod
                try:
                    from trn_agent_boot.trn_boot import (
                        _ntff_profile_via_ctypes)
                    mod.set_axon_ntff_profile_hook(
                        _ntff_profile_via_ctypes("/opt/axon/libaxon_pjrt.so"))
                except Exception:
                    pass
        import concourse.bass_utils as _bu
        _orig_upload = _bu.upload_artifacts

        def _safe_upload(tmpdir):
            try:
                return _orig_upload(tmpdir)
            except Exception:
                return f"file://{tmpdir}"

        _bu.upload_artifacts = _safe_upload
    except Exception:
        pass


_install_profile_shim()


def _ensure_device_healthy():
    """A crashed prior run can leave the tunneled NeuronCores in
    NRT_EXEC_UNIT_UNRECOVERABLE; axon_reset() recovers them. Probe with a
    tiny transfer and reset once if it fails. Never raises."""
    try:
        import ctypes
        import jax
        devs = jax.devices()
        try:
            np.asarray(jax.device_put(np.ones(2, np.float32), devs[0]))
            return
        except Exception:
            pass
        lib = ctypes.CDLL("/opt/axon/libaxon_pjrt.so")
        lib.axon_reset.restype = ctypes.c_int64
        lib.axon_reset()
    except Exception:
        pass


N_CORES = 8
B = 65536
B_LOCAL = B // N_CORES
SIZES = (1024, 512, 768)
OUT = 512
RANK = 10
SUPER = 512  # batch rows per supertile
F32 = mybir.dt.float32
FP16 = mybir.dt.float16


def build(b_local=B_LOCAL, num_devices=1, xin_bufs=6, xt_bufs=14,
          pst_bufs=5, lag=2):
    nsup = b_local // SUPER
    kts = [f // 128 for f in SIZES]  # k-tiles per input: 8, 4, 6
    groups = [(i, kt) for i in range(3) for kt in range(kts[i])]  # 18

    nc = bacc.Bacc("TRN2", target_bir_lowering=False, debug=False,
                   num_devices=num_devices)
    x_dram = [
        nc.dram_tensor(f"x{i+1}", (b_local, SIZES[i]), FP16,
                       kind="ExternalInput").ap()
        for i in range(3)
    ]
    f_dram = [
        nc.dram_tensor(f"f{i+1}", (SIZES[i], RANK), FP16,
                       kind="ExternalInput").ap()
        for i in range(3)
    ]
    fo_dram = nc.dram_tensor("f_out", (OUT, RANK), FP16,
                             kind="ExternalInput").ap()
    out_dram = nc.dram_tensor("out", (b_local, OUT), FP16,
                              kind="ExternalOutput").ap()

    with tile.TileContext(nc) as tc:
        with (
            tc.tile_pool(name="const", bufs=1) as constp,
            tc.tile_pool(name="xin", bufs=xin_bufs) as xinp,
            tc.tile_pool(name="xt", bufs=xt_bufs) as xtp,
            tc.tile_pool(name="ysb", bufs=2) as yp,
            tc.tile_pool(name="osb", bufs=2) as osp,
            tc.tile_pool(name="pst", bufs=pst_bufs, space="PSUM") as pst,
            tc.tile_pool(name="psy", bufs=1, space="PSUM") as psy,
            tc.tile_pool(name="pso", bufs=2, space="PSUM") as pso,
        ):
            in_pat = "(s p blk) f -> s p blk f"
            out_pat = "(s p blk) o -> s p blk o"

            def emit_loads(s):
                x_t = []
                for i in range(3):
                    t = xinp.tile([128, 4, SIZES[i]], FP16, tag=f"x{i}",
                                  name=f"x_t{i}_{s}")
                    src = x_dram[i].rearrange(in_pat, blk=4, p=128)[s]
                    nc.sync.dma_start(t[:], src)
                    x_t.append(t)
                return x_t

            # factor matrices first (tiny), then the first supertiles of
            # x, so the PE's first real work isn't serialized behind the
            # bulk queue
            f_sb = []
            for i in range(3):
                t = constp.tile([128, kts[i], RANK], FP16, tag=f"f{i}",
                                name=f"f_sb{i}")
                nc.sync.dma_start(
                    t[:], f_dram[i].rearrange("(kt p) r -> p kt r", p=128))
                f_sb.append(t)
            fo_sb = constp.tile([128, 4, RANK], FP16, tag="fo")
            nc.sync.dma_start(
                fo_sb[:], fo_dram.rearrange("(blk p) r -> p blk r", p=128))
            preloaded = {0: emit_loads(0), 1: emit_loads(1)}

            # identity for PE transposes
            ident = constp.tile([128, 128], FP16)
            make_identity(nc, ident[:])

            # f_outT [10, 4, 128] fp16 via 4 PE transposes (regular
            # matmul against identity)
            foT_ps = pso.tile([RANK, 4, 128], F32, tag="ops",
                              name="foT_ps")
            for blk in range(4):
                nc.tensor.matmul(foT_ps[:, blk, :], fo_sb[:, blk, :],
                                 ident[:], start=True, stop=True)
            foT = constp.tile([RANK, 4, 128], FP16, tag="foT")
            nc.scalar.copy(foT[:], foT_ps[:])

            # rotation of PSUM->SBUF drain engines for the big copies
            def v_copy(dst, src):
                nc.vector.tensor_copy(dst, src)

            def s_copy(dst, src):
                nc.scalar.copy(dst, src)

            # GPSIMD cannot access PSUM on TRN2, so only DVE + Act can
            # drain; Act also carries y2/hT copies + store triggers, so it
            # gets slightly fewer of the big drains
            # 10 DVE / 8 Act per 18 drains: Act also carries y2/hT + the
            # store trigger
            xt_rr = itertools.cycle([v_copy, s_copy, v_copy, s_copy,
                                     v_copy, s_copy, v_copy, s_copy,
                                     v_copy])
            o_rr = itertools.cycle([v_copy, s_copy])

            def emit_transpose_group(st, gi):
                i, kt = groups[gi]
                s = st["s"]
                xT_ps = pst.tile([128, SUPER], F32, tag="xtps",
                                 name=f"xtps_{s}_{gi}")
                for blk in range(4):
                    nc.tensor.matmul(
                        xT_ps[:, blk * 128:(blk + 1) * 128],
                        st["x_t"][i][:, blk, kt * 128:(kt + 1) * 128],
                        ident[:], start=True, stop=True)
                xT_sb = xtp.tile([128, SUPER], FP16, tag="xtsb",
                                 name=f"xtsb_{s}_{gi}")
                next(xt_rr)(xT_sb[:], xT_ps[:])
                st["xT"][gi] = xT_sb

            def emit_chains(st, i):
                # One (i, blk) accumulation chain at a time: PSUM start=True
                # clears the bank's has_written bits bank-wide, so chains
                # sharing the y bank must not interleave their start..stop
                # windows (interleaving loses each sibling's first k-tile).
                for blk in range(4):
                    for kt in range(kts[i]):
                        xT_sb = st["xT"][sum(kts[:i]) + kt]
                        nc.tensor.matmul(
                            st["y_ps"][:, i, blk, :],
                            xT_sb[:, blk * 128:(blk + 1) * 128],
                            f_sb[i][:, kt, :],
                            start=(kt == 0), stop=(kt == kts[i] - 1))

            def emit_y2(st):
                # y_ps[:, 1] is final once input-1 chains are done; copy
                # it out early so the Hadamard muls don't wait on Act
                y2 = yp.tile([128, 4, RANK], FP16, tag="y2",
                             name=f"y2_{st['s']}")
                nc.scalar.copy(y2[:], st["y_ps"][:, 1])
                st["y2"] = y2

            def emit_muls(st):
                y_ps = st["y_ps"]
                h = yp.tile([128, 4, RANK], FP16, tag="h",
                            name=f"h_{st['s']}")
                nc.vector.tensor_mul(h[:], y_ps[:, 0], st["y2"][:])
                nc.vector.tensor_mul(h[:], h[:], y_ps[:, 2])
                st["h"] = h

            def emit_hT(st):
                s = st["s"]
                hT_ps = pso.tile([RANK, 4, 128], F32, tag="ops",
                                 name=f"htps_{s}")
                for blk in range(4):
                    nc.tensor.matmul(hT_ps[:, blk, :], st["h"][:, blk, :],
                                     ident[:], start=True, stop=True)
                hT = yp.tile([RANK, 4, 128], FP16, tag="ht",
                             name=f"ht_{s}")
                nc.scalar.copy(hT[:], hT_ps[:])
                st["hT"] = hT

            def emit_final(st):
                s = st["s"]
                o_sb = osp.tile([128, 4, OUT], FP16, tag="osb",
                                name=f"osb_{s}")
                for blk in range(4):
                    o_ps = pso.tile([128, OUT], F32, tag="ops",
                                    name=f"ops_{s}_{blk}")
                    nc.tensor.matmul(o_ps[:], st["hT"][:, blk, :], foT[:],
                                     start=True, stop=True)
                    next(o_rr)(o_sb[:, blk, :], o_ps[:])
                dst = out_dram.rearrange(out_pat, blk=4, p=128)[s]
                nc.scalar.dma_start(dst, o_sb[:])

            # Emission schedule per supertile s (PE program order):
            #   transposes g0..g17 interleaved with, at fixed offsets:
            #     g1:  k-chains for input 2 of s-1 + Hadamard muls
            #     g3:  s-1 Hadamard transpose (+ its Act drain)
            #     g5:  s-1 final matmuls + store
            #     g12: k-chains for input 0 of s (copies g0..g7 drained)
            #     g16: k-chains for input 1 of s, then early y2 copy
            # so the PE never waits on a PSUM->SBUF copy round-trip.
            prev = None
            for s in range(nsup):
                st = {"s": s, "xT": {}}
                st["x_t"] = preloaded.pop(s, None) or emit_loads(s)
                st["y_ps"] = psy.tile([128, 3, 4, RANK], F32, tag="y",
                                      name=f"y_ps_{s}")
                for gi in range(len(groups)):
                    emit_transpose_group(st, gi)
                    if prev is not None:
                        if gi == 1:
                            emit_chains(prev, 2)
                            emit_muls(prev)
                        elif gi == 3:
                            emit_hT(prev)
                        elif gi == 5:
                            emit_final(prev)
                            prev = None
                    if gi == 12:
                        emit_chains(st, 0)
                    if gi == 16:
                        emit_chains(st, 1)
                        emit_y2(st)
                prev = st
            emit_chains(prev, 2)
            emit_muls(prev)
            emit_hT(prev)
            emit_final(prev)

    nc.compile()
    return nc


_NC_CACHE = {}


def _get_nc(b_local=B_LOCAL):
    if b_local not in _NC_CACHE:
        _NC_CACHE[b_local] = build(b_local)
    return _NC_CACHE[b_local]


LAST_RESULT = None


def kernel(x1, x2, x3, f1, f2, f3, f_out, _trace=False, _tmpdir=None):
    global LAST_RESULT
    _ensure_device_healthy()
    x1, x2, x3 = (np.asarray(a) for a in (x1, x2, x3))
    f16 = [np.ascontiguousarray(np.asarray(a), dtype=np.float32)
           .astype(np.float16) for a in (f1, f2, f3, f_out)]
    nc = _get_nc()
    in_maps = []
    for c in range(N_CORES):
        sl = slice(c * B_LOCAL, (c + 1) * B_LOCAL)
        in_maps.append({
            "x1": x1[sl].astype(np.float16),
            "x2": x2[sl].astype(np.float16),
            "x3": x3[sl].astype(np.float16),
            "f1": f16[0], "f2": f16[1], "f3": f16[2], "f_out": f16[3],
        })
    kw = {}
    if _trace:
        kw = {"trace": True, "tmpdir": _tmpdir}
    res = run_bass_kernel_spmd(nc, in_maps, core_ids=list(range(N_CORES)),
                               **kw)
    LAST_RESULT = res
    return np.concatenate(
        [res.results[c]["out"] for c in range(N_CORES)],
        axis=0).astype(np.float32)
